# revision 1
# baseline (speedup 1.0000x reference)
"""Trainium2 Bass kernel: differentiable-optics PSF (batch=128, 2 focus, 3 ch).

Math: per (b, f, i): pupil = A * exp(i*2pi/lam * (O_f + delta_bf * Z4)).
Z4 = sqrt(3)(2x^2 + 2y^2 - 1) is separable -> exp term = g(y) g(x) outer
product, g complex 256-vec.  So pupil = diag(g) Q diag(g) with
Q_{i,f} = A*exp(i k_i O_f) precomputed.  Shifted-DFT + 32x32 bilinear
sampling only need 64 rows/cols C_i of the field:
  field^T = S^T M1^T accumulated in PSUM, M1 = S^T Q, S = diag(g) F[:,C].
Sampling: psf^T -> free-dim 2-tap blend -> W matmul -> [32,32]^T,
batched normalize + 32x32 block transpose at the end.
"""
import numpy as np
from math import factorial

GRID = 256
FOV = 32
NZ = 15
F_MM = 25.0
F_NUMBER = 2.0
PIXEL_SIZE = 3.45e-6
F_M = F_MM * 1e-3
PUPIL_DIAM = F_M / F_NUMBER
BATCH = 128
NCORES = 8
BPC = BATCH // NCORES          # batch per core
NIMG = BPC * 2 * 3             # images per core (b, f, i)


def _noll_to_nm(j):
    n = 0
    k = j - 1
    while k > n:
        n += 1
        k -= n
    m = (-1) ** j * ((n % 2) + 2 * ((k + ((n + 1) % 2)) // 2))
    return n, m


def _zernike(n, m, r, theta):
    am = abs(m)
    R = np.zeros_like(r)
    for s in range((n - am) // 2 + 1):
        c = ((-1) ** s * factorial(n - s)
             / (factorial(s) * factorial((n + am) // 2 - s)
                * factorial((n - am) // 2 - s)))
        R = R + c * r ** (n - 2 * s)
    norm = np.sqrt(n + 1) if m == 0 else np.sqrt(2 * (n + 1))
    ang = np.cos(am * theta) if m >= 0 else np.sin(am * theta)
    return np.where(r <= 1.0, norm * R * ang, 0.0)


def _host_consts(wavelengths):
    """Input-independent structural constants (DFT/sampling matrices)."""
    N = GRID
    # shifted DFT: field = Ft X Ft, Ft[a,b] = F[(a+128)%256,(b+128)%256]
    idx = (np.arange(N) + N // 2) % N
    jk = np.outer(idx, idx).astype(np.float64)
    ang = -2.0 * np.pi * jk / N
    Fr_full = np.cos(ang)
    Fi_full = np.sin(ang)

    csel = np.zeros((3, 64), np.int64)      # tap rows per channel
    wt0 = np.zeros((3, 32), np.float32)     # tap weights
    wt1 = np.zeros((3, 32), np.float32)
    for i in range(3):
        lam = float(wavelengths[i])
        zoom = PIXEL_SIZE * FOV * PUPIL_DIAM / (lam * F_M * GRID)
        g1 = (np.arange(FOV, dtype=np.float32) / np.float32(FOV - 1)
              * np.float32(2.0 * zoom) - np.float32(zoom))
        x = ((g1 + 1.0) * GRID - 1.0) * 0.5
        x0 = np.floor(x)
        tx = (x - x0).astype(np.float32)
        x0 = x0.astype(np.int64)
        csel[i, 0::2] = x0
        csel[i, 1::2] = x0 + 1
        wt0[i] = 1.0 - tx
        wt1[i] = tx
    return Fr_full, Fi_full, csel, wt0, wt1

def build_nc():
    import concourse.bass as bass
    import concourse.bacc as bacc
    import concourse.mybir as mybir
    from concourse.tile import TileContext

    f32 = mybir.dt.float32
    f32r = mybir.dt.float32r
    bf16 = mybir.dt.bfloat16
    AF = mybir.ActivationFunctionType
    OP = mybir.AluOpType
    TWO_PI = float(2.0 * np.pi)

    nc = bacc.Bacc("TRN2", target_bir_lowering=False)
    # device inputs (per core)
    qrd = nc.declare_dram_parameter("qr", [128, 6 * 512], bf16, isOutput=False)
    qid = nc.declare_dram_parameter("qi", [128, 6 * 512], bf16, isOutput=False)
    xad = nc.declare_dram_parameter("xa", [128, 3 * 512], f32, isOutput=False)
    xbd = nc.declare_dram_parameter("xb", [128, 3 * 512], f32, isOutput=False)
    wvd = nc.declare_dram_parameter("wv", [1, 256], f32, isOutput=False)
    erowd = nc.declare_dram_parameter("erow", [1, NIMG], f32, isOutput=False)
    wtd = nc.declare_dram_parameter("wt", [64, 3 * 32], f32, isOutput=False)
    w0d = nc.declare_dram_parameter("w0m", [64, 3 * 32], f32, isOutput=False)
    w1d = nc.declare_dram_parameter("w1m", [64, 3 * 32], f32, isOutput=False)
    onesd = nc.declare_dram_parameter("ones32", [32, 1], f32, isOutput=False)
    outd = nc.declare_dram_parameter("out", [NIMG, 32, 32], f32, isOutput=True)

    with TileContext(nc) as tc:
        with (
            tc.tile_pool(name="const", bufs=1) as cpool,
            tc.tile_pool(name="g", bufs=1) as gpool,
            tc.tile_pool(name="s", bufs=10) as spool,
            tc.tile_pool(name="m1b", bufs=10) as mpool,
            tc.tile_pool(name="m1t", bufs=10) as tpool,
            tc.tile_pool(name="psf", bufs=10) as fpool,
            tc.tile_pool(name="fin", bufs=1) as opool,
            tc.tile_pool(name="ps1", bufs=3, space="PSUM") as ps1,
            tc.tile_pool(name="ps2", bufs=2, space="PSUM") as ps2,
            tc.tile_pool(name="ps3", bufs=1, space="PSUM") as ps3,
            tc.tile_pool(name="psg", bufs=1, space="PSUM") as psg,
        ):
            # ---- load constants ----
            qr = cpool.tile([128, 6 * 512], bf16, tag="qr")
            qi = cpool.tile([128, 6 * 512], bf16, tag="qi")
            nc.sync.dma_start(qr[:], qrd[:])
            nc.sync.dma_start(qi[:], qid[:])
            xa = cpool.tile([128, 3 * 512], f32, tag="xa")
            xb = cpool.tile([128, 3 * 512], f32, tag="xb")
            nc.sync.dma_start(xa[:], xad[:])
            nc.sync.dma_start(xb[:], xbd[:])
            wv0 = cpool.tile([1, 256], f32, tag="wv0")
            erow0 = cpool.tile([1, NIMG], f32, tag="erow0")
            nc.sync.dma_start(wv0[:], wvd[:])
            nc.sync.dma_start(erow0[:], erowd[:])
            wt0_ = cpool.tile([64, 3 * 32], f32, tag="wt0_")
            w0m = cpool.tile([64, 3 * 32], f32, tag="w0m")
            w1m = cpool.tile([64, 3 * 32], f32, tag="w1m")
            nc.sync.dma_start(wt0_[:], wtd[:])
            nc.sync.dma_start(w0m[:], w0d[:])
            nc.sync.dma_start(w1m[:], w1d[:])
            ones0 = cpool.tile([32, 1], f32, tag="ones0")
            nc.sync.dma_start(ones0[:], onesd[:])
            # matmul operands must come from ONE producer sem (LDW wait limit)
            wv = cpool.tile([1, 256], f32, tag="wv")
            erow = cpool.tile([1, NIMG], f32, tag="erow")
            wt = cpool.tile([64, 3 * 32], f32, tag="wt")
            ones32 = cpool.tile([32, 1], f32, tag="ones32")
            nc.vector.tensor_copy(wv[:], wv0[:])
            nc.vector.tensor_copy(erow[:], erow0[:])
            nc.vector.tensor_copy(wt[:], wt0_[:])
            nc.vector.tensor_copy(ones32[:], ones0[:])

            # ---- batched g vectors: gcos/gsin [128, 2*NIMG], col = t*NIMG+j
            gcos = gpool.tile([128, 2 * NIMG], f32, tag="gcos")
            gsin = gpool.tile([128, 2 * NIMG], f32, tag="gsin")
            for t in range(2):
                pg = psg.tile([128, NIMG], f32, tag="pg")
                nc.tensor.matmul(pg[:], wv[0:1, t * 128:(t + 1) * 128],
                                 erow[0:1, :], start=True, stop=True)
                gm = gpool.tile([128, NIMG], f32, tag="gm")
                gmc = gpool.tile([128, NIMG], f32, tag="gmc")
                ua = gpool.tile([128, NIMG], f32, tag="ua")
                ub = gpool.tile([128, NIMG], f32, tag="ub")
                nc.vector.tensor_scalar_add(ua[:], pg[:], 256.0)
                nc.vector.tensor_scalar_add(ub[:], pg[:], 256.25)
                ui = gpool.tile([128, NIMG], mybir.dt.int32, tag="ui")
                uf = gpool.tile([128, NIMG], f32, tag="uf")
                nc.vector.tensor_copy(ui[:], ua[:])
                nc.vector.tensor_copy(uf[:], ui[:])
                nc.vector.tensor_sub(gm[:], ua[:], uf[:])
                nc.vector.tensor_copy(ui[:], ub[:])
                nc.vector.tensor_copy(uf[:], ui[:])
                nc.vector.tensor_sub(gmc[:], ub[:], uf[:])
                sl = slice(t * NIMG, (t + 1) * NIMG)
                nc.scalar.activation(gsin[:, sl], gm[:], AF.Sin, scale=TWO_PI)
                nc.scalar.activation(gcos[:, sl], gmc[:], AF.Sin, scale=TWO_PI)

            o_all = opool.tile([32, NIMG * 32], f32, tag="o_all")

            # ---- per-image pipeline ----
            for j in range(NIMG):
                f = j // (BPC * 3)
                i = j % 3
                p6 = f * 3 + i
                # fused build: s_cat[:, t*256:(t+1)*256] = [S1_t | S2_t]
                s_cat = spool.tile([128, 512], bf16, tag="s_cat")
                t1 = spool.tile([128, 512], f32, tag="t1")
                for t in range(2):
                    gc = gcos[:, t * NIMG + j: t * NIMG + j + 1]
                    gs = gsin[:, t * NIMG + j: t * NIMG + j + 1]
                    osl = slice(t * 256, (t + 1) * 256)
                    xsl = slice(i * 512 + t * 256, i * 512 + (t + 1) * 256)
                    nc.vector.tensor_scalar_mul(t1[:, osl], xa[:, xsl], gc)
                    nc.vector.scalar_tensor_tensor(
                        s_cat[:, osl], xb[:, xsl], gs, t1[:, osl],
                        op0=OP.mult, op1=OP.add)
                # stage 1: psum [M1r; M1i] = [S1|S2]^T [Qr;Qi]
                pm1 = ps1.tile([128, 256], f32, tag="pm1")
                for t in range(2):
                    qsl = slice(p6 * 512 + t * 256, p6 * 512 + (t + 1) * 256)
                    nc.tensor.matmul(pm1[:], s_cat[:, t * 256: t * 256 + 128],
                                     qr[:, qsl], start=(t == 0), stop=False)
                for t in range(2):
                    qsl = slice(p6 * 512 + t * 256, p6 * 512 + (t + 1) * 256)
                    nc.tensor.matmul(pm1[:], s_cat[:, t * 256 + 128:
                                     (t + 1) * 256],
                                     qi[:, qsl], start=False, stop=(t == 1))
                # cast to bf16 then DMA-transpose halves into [256,64] tiles
                m1b = mpool.tile([128, 256], bf16, tag="m1b")
                nc.scalar.copy(m1b[:], pm1[:])
                # m1t[:, c, 0:64]=M1rT_c ; m1t[:, c, 64:128]=M1iT_c
                m1t = tpool.tile([128, 256], bf16, tag="m1t")
                nc.sync.dma_start_transpose(
                    m1t[:].rearrange("p (c r) -> p c r", c=2), m1b[:])
                # stage 2: psum [fieldR^T; fieldI^T]
                pm2 = ps2.tile([128, 64], f32, tag="pm2")
                for c in range(2):
                    nc.tensor.matmul(pm2[:], s_cat[:, c * 256: c * 256 + 128],
                                     m1t[:, c * 128: c * 128 + 64],
                                     start=(c == 0), stop=False)
                for c in range(2):
                    nc.tensor.matmul(pm2[:], s_cat[:, c * 256 + 128:
                                     (c + 1) * 256],
                                     m1t[:, c * 128 + 64: (c + 1) * 128],
                                     start=False, stop=(c == 1))
                # psf^T = fr^2 + fi^2  [64(c), 64(y)]
                sqr = fpool.tile([64, 64], f32, tag="sqr")
                sqi = fpool.tile([64, 64], f32, tag="sqi")
                psfT = fpool.tile([64, 64], f32, tag="psfT")
                nc.scalar.activation(sqr[:], pm2[0:64, :], AF.Square)
                nc.scalar.activation(sqi[:], pm2[64:128, :], AF.Square)
                nc.gpsimd.tensor_add(psfT[:], sqr[:], sqi[:])
                # y-side 2-tap blend -> A1 [64(c), 32]
                ea = fpool.tile([64, 32], f32, tag="ea")
                eb = fpool.tile([64, 32], f32, tag="eb")
                a1 = fpool.tile([64, 32], f32, tag="a1")
                wsl = slice(i * 32, (i + 1) * 32)
                psfT3 = psfT[:].rearrange("p (a two) -> p a two", two=2)
                nc.gpsimd.tensor_mul(ea[:], psfT3[:, :, 0], w0m[:, wsl])
                nc.gpsimd.tensor_mul(eb[:], psfT3[:, :, 1], w1m[:, wsl])
                nc.gpsimd.tensor_add(a1[:], ea[:], eb[:])
                # x-side via matmul: O = W a1 -> [32(q), 32(p)] = out32^T
                pm3 = ps3.tile([32, 32], f32, tag="pm3")
                nc.tensor.matmul(pm3[:], wt[:, wsl], a1[:],
                                 start=True, stop=True)
                nc.vector.tensor_copy(o_all[:, j * 32:(j + 1) * 32],
                                      pm3[:])

            # ---- batched finalize: sums, normalize, block-transpose, out
            o3 = o_all[:].rearrange("q (j p) -> q j p", p=32)
            csum = opool.tile([32, NIMG], f32, tag="csum")
            nc.vector.tensor_reduce(csum[:], o3, op=OP.add,
                                    axis=mybir.AxisListType.X)
            pcs = psg.tile([1, NIMG], f32, tag="pcs")
            nc.tensor.matmul(pcs[:], ones32[:], csum[:], start=True, stop=True)
            rec = opool.tile([1, NIMG], f32, tag="rec")
            nc.vector.tensor_scalar_add(rec[:], pcs[:], 1e-8)
            nc.vector.reciprocal(rec[:], rec[:])
            recb = opool.tile([32, NIMG], f32, tag="recb")
            nc.gpsimd.partition_broadcast(recb[:], rec[:])
            t_all = opool.tile([32, NIMG * 32], f32, tag="t_all")
            nc.vector.transpose(t_all[:], o_all[:])
            for j in range(NIMG):
                jsl = slice(j * 32, (j + 1) * 32)
                nc.vector.tensor_scalar_mul(t_all[:, jsl], t_all[:, jsl],
                                            recb[:, j:j + 1])
            nc.sync.dma_start(outd[:].rearrange("j p q -> p j q"),
                              t_all[:].rearrange("p (j q) -> p j q", q=32))
    nc.compile()
    return nc


_CACHE = {}


def _get_nc():
    if "nc" not in _CACHE:
        _CACHE["nc"] = build_nc()
    return _CACHE["nc"]

def kernel(d_obj, current_focus_dist_0, current_focus_dist_90,
           zernike_0, zernike_90, zernike_basis, aperture, wavelengths):
    from concourse.bass_utils import run_bass_kernel_spmd

    d_obj = np.asarray(d_obj, np.float32)
    zernike_0 = np.asarray(zernike_0, np.float32)
    zernike_90 = np.asarray(zernike_90, np.float32)
    basis = np.asarray(zernike_basis, np.float32)
    aperture = np.asarray(aperture, np.float32)
    lam = np.asarray(wavelengths, np.float32)
    f0 = float(current_focus_dist_0)
    f90 = float(current_focus_dist_90)

    Fr_full, Fi_full, csel, wt0, wt1 = _host_consts(lam)

    # Q planes: A * exp(i * 2pi/lam * O_f) ; O_f = sum_n z_f[n] basis_n
    O = np.tensordot(np.stack([zernike_0, zernike_90]),
                     basis.reshape(NZ, -1), axes=[[1], [0]])  # [2, 65536]
    import ml_dtypes
    bf = ml_dtypes.bfloat16
    O = O.reshape(2, GRID, GRID).astype(np.float64)
    qr = np.empty((128, 6 * 512), bf)
    qi = np.empty((128, 6 * 512), bf)
    for f in range(2):
        for i in range(3):
            ph = 2.0 * np.pi * O[f] / float(lam[i])
            Qr = (aperture * np.cos(ph)).astype(bf)
            Qi = (aperture * np.sin(ph)).astype(bf)
            p6 = f * 3 + i
            for t in range(2):
                sl = slice(p6 * 512 + t * 256, p6 * 512 + (t + 1) * 256)
                qr[:, sl] = Qr[t * 128:(t + 1) * 128]
                qi[:, sl] = Qi[t * 128:(t + 1) * 128]

    # concatenated consts: XA=[X1|X1s], XB=[X1s|X2b], [128, ch*2tile*256]
    xa = np.empty((128, 3 * 512), np.float32)
    xb = np.empty((128, 3 * 512), np.float32)
    for i in range(3):
        Fcr = Fr_full[:, csel[i]].astype(np.float32)   # [256, 64]
        Fci = Fi_full[:, csel[i]].astype(np.float32)
        X1 = np.concatenate([Fcr, Fci], 1)             # [256,128]
        X1s = np.concatenate([-Fci, Fcr], 1)
        X2b = np.concatenate([-Fcr, -Fci], 1)
        XA = np.concatenate([X1, X1s], 1)              # [256,256]
        XB = np.concatenate([X1s, X2b], 1)
        for t in range(2):
            sl = slice(i * 512 + t * 256, i * 512 + (t + 1) * 256)
            xa[:, sl] = XA[t * 128:(t + 1) * 128]
            xb[:, sl] = XB[t * 128:(t + 1) * 128]

    # wv row: 2*v^2 - 0.5 on the [-1,1] grid
    lin = np.linspace(-1.0, 1.0, GRID)
    wv = (2.0 * lin * lin - 0.5).astype(np.float32)[None, :]

    # W^T and tap-weight mats (row-replicated), [64, 3*32]
    wt = np.zeros((64, 3 * 32), np.float32)
    w0m = np.zeros((64, 3 * 32), np.float32)
    w1m = np.zeros((64, 3 * 32), np.float32)
    for i in range(3):
        W = np.zeros((32, 64), np.float32)
        for p in range(32):
            W[p, 2 * p] = wt0[i, p]
            W[p, 2 * p + 1] = wt1[i, p]
        wt[:, i * 32:(i + 1) * 32] = W.T
        w0m[:, i * 32:(i + 1) * 32] = np.tile(wt0[i][None, :], (64, 1))
        w1m[:, i * 32:(i + 1) * 32] = np.tile(wt1[i][None, :], (64, 1))

    # defocus scalars -> e_j = delta * sqrt(3) / lam
    delta = np.stack([
        F_M ** 2 / (8.0 * F_NUMBER ** 2) * (1.0 / f0 - 1.0 / (d_obj + 1e-8)),
        F_M ** 2 / (8.0 * F_NUMBER ** 2) * (1.0 / f90 - 1.0 / (d_obj + 1e-8)),
    ])  # [2, 128]

    nc = _get_nc()
    in_maps = []
    for c in range(NCORES):
        erow = np.empty((1, NIMG), np.float32)
        for j in range(NIMG):
            f = j // (BPC * 3)
            b = (j // 3) % BPC
            i = j % 3
            erow[0, j] = delta[f, c * BPC + b] * np.sqrt(3.0) / float(lam[i])
        in_maps.append({
            "qr": qr, "qi": qi, "xa": xa, "xb": xb,
            "wv": wv, "erow": erow, "wt": wt, "w0m": w0m, "w1m": w1m,
            "ones32": np.ones((32, 1), np.float32),
        })
    trace = bool(_CACHE.get("trace"))
    res = run_bass_kernel_spmd(nc, in_maps, list(range(NCORES)), trace=trace)
    _CACHE["last_res"] = res
    outs = res.results
    psf0 = np.empty((BATCH, 3, FOV, FOV), np.float32)
    psf90 = np.empty((BATCH, 3, FOV, FOV), np.float32)
    for c in range(NCORES):
        o = np.asarray(outs[c]["out"]).reshape(2, BPC, 3, FOV, FOV)
        psf0[c * BPC:(c + 1) * BPC] = o[0]
        psf90[c * BPC:(c + 1) * BPC] = o[1]
    return psf0, psf90



# revision 2
# speedup vs baseline: 2.0481x; 2.0481x over previous
"""Trainium2 Bass kernel: differentiable-optics PSF (batch=128, 2 focus, 3 ch).

Math per image (b, f, i):  pupil = diag(g) Q diag(g),  Q = A*exp(i*2pi*O_f/lam)
precomputed on host; g(v) = exp(i*2pi*e*w(v)) the separable defocus chirp.
field needed only at 64x64 taps (bilinear sampling of |field|^2):
  stage1  M = Q^T S   (S = diag(g) Fs[:,taps], columns pre-scaled by
                       sqrt(bilinear weight)/16 on host -> blend mults vanish)
  stage2  field^T = [Sr|Si]-combos^T M   (s-taps on partitions)
  |.|^2 -> pair-add (r-side) -> 0/1 sampling matmul folds r/i sum + s-side
  pairs -> [32,32].  Normalization + final transpose on host.
Engines: PE matmuls; DVE 4x-mode fp16 tensor_scalar builds; Pool adds;
Act psum->fp16 copies + squares.  No DMA transpose, no per-image DVE
normalize.
"""
import numpy as np

GRID = 256
FOV = 32
NZ = 15
F_MM = 25.0
F_NUMBER = 2.0
PIXEL_SIZE = 3.45e-6
F_M = F_MM * 1e-3
PUPIL_DIAM = F_M / F_NUMBER
BATCH = 128
NCORES = 8
BPC = BATCH // NCORES          # batch per core
NIMG = BPC * 2 * 3             # images per core, jj = (f*3+i)*16 + b
NGRP = NIMG // 4               # psum groups of 4 images
SCALE = 1.0 / 16.0             # per-side amplitude scale (fp16 range)


def _host_consts(lam):
    """Input-independent tap/weight constants. Taps in split order:
    cols 0:32 = x0 taps, cols 32:64 = x0+1 taps."""
    csel = np.zeros((3, 64), np.int64)
    wroot = np.zeros((3, 64), np.float64)
    for i in range(3):
        zoom = PIXEL_SIZE * FOV * PUPIL_DIAM / (float(lam[i]) * F_M * GRID)
        g1 = (np.arange(FOV, dtype=np.float32) / np.float32(FOV - 1)
              * np.float32(2.0 * zoom) - np.float32(zoom))
        x = ((g1 + 1.0) * GRID - 1.0) * 0.5
        x0 = np.floor(x)
        tx = (x - x0).astype(np.float64)
        csel[i, 0:32] = x0.astype(np.int64)
        csel[i, 32:64] = x0.astype(np.int64) + 1
        wroot[i, 0:32] = np.sqrt(1.0 - tx) * SCALE
        wroot[i, 32:64] = np.sqrt(tx) * SCALE
    return csel, wroot


def build_nc():
    import concourse.bass as bass
    import concourse.bacc as bacc
    import concourse.mybir as mybir
    from concourse.tile import TileContext

    f32 = mybir.dt.float32
    fp16 = mybir.dt.float16
    i32 = mybir.dt.int32
    AF = mybir.ActivationFunctionType
    OP = mybir.AluOpType
    TWO_PI = float(2.0 * np.pi)

    nc = bacc.Bacc("TRN2", target_bir_lowering=False)
    qrd = nc.declare_dram_parameter("qr", [128, 24 * 128], fp16, isOutput=False)
    qid = nc.declare_dram_parameter("qi", [128, 24 * 128], fp16, isOutput=False)
    xamd = nc.declare_dram_parameter("xam", [128, 6 * 128], fp16, isOutput=False)
    xbmd = nc.declare_dram_parameter("xbm", [128, 6 * 128], fp16, isOutput=False)
    wserd = nc.declare_dram_parameter("wser", [128, 3 * 32], fp16, isOutput=False)
    erowd = nc.declare_dram_parameter("erow", [1, NIMG], f32, isOutput=False)
    wvd = nc.declare_dram_parameter("wv", [1, 256], f32, isOutput=False)
    outd = nc.declare_dram_parameter("out", [32, NIMG * 32], f32, isOutput=True)

    with TileContext(nc) as tc:
        with (
            tc.tile_pool(name="const", bufs=1) as cpool,
            tc.tile_pool(name="g", bufs=1) as gpool,
            tc.tile_pool(name="m", bufs=5) as mpool,
            tc.tile_pool(name="sc", bufs=12) as scpool,
            tc.tile_pool(name="m1", bufs=3) as m1pool,
            tc.tile_pool(name="sq", bufs=3) as sqpool,
            tc.tile_pool(name="a1", bufs=2) as a1pool,
            tc.tile_pool(name="fin", bufs=1) as opool,
            tc.tile_pool(name="ps1", bufs=2, space="PSUM") as ps1,
            tc.tile_pool(name="ps2", bufs=2, space="PSUM") as ps2,
            tc.tile_pool(name="ps3", bufs=1, space="PSUM") as ps3,
            tc.tile_pool(name="psg", bufs=1, space="PSUM") as psg,
        ):
            # ---- load constants ----
            qr = cpool.tile([128, 24 * 128], fp16, tag="qr")
            qi = cpool.tile([128, 24 * 128], fp16, tag="qi")
            nc.sync.dma_start(qr[:], qrd[:])
            nc.sync.dma_start(qi[:], qid[:])
            xam = cpool.tile([128, 6 * 128], fp16, tag="xam")
            xbm = cpool.tile([128, 6 * 128], fp16, tag="xbm")
            nc.sync.dma_start(xam[:], xamd[:])
            nc.sync.dma_start(xbm[:], xbmd[:])
            wser0 = cpool.tile([128, 3 * 32], fp16, tag="wser0")
            nc.sync.dma_start(wser0[:], wserd[:])
            wv0 = cpool.tile([1, 256], f32, tag="wv0")
            erow0 = cpool.tile([1, NIMG], f32, tag="erow0")
            nc.sync.dma_start(wv0[:], wvd[:])
            nc.sync.dma_start(erow0[:], erowd[:])
            # single-producer copies for the tiny outer-product matmul
            wv = cpool.tile([1, 256], f32, tag="wv")
            erow = cpool.tile([1, NIMG], f32, tag="erow")
            wser = cpool.tile([128, 3 * 32], fp16, tag="wser")
            nc.vector.tensor_copy(wv[:], wv0[:])
            nc.vector.tensor_copy(erow[:], erow0[:])
            nc.vector.tensor_copy(wser[:], wser0[:])

            # ---- per-image chirp factors gcos/gsin [128, 2*NIMG] ----
            gcos = gpool.tile([128, 2 * NIMG], f32, tag="gcos")
            gsin = gpool.tile([128, 2 * NIMG], f32, tag="gsin")
            for t in range(2):
                pg = psg.tile([128, NIMG], f32, tag="pg")
                nc.tensor.matmul(pg[:], wv[0:1, t * 128:(t + 1) * 128],
                                 erow[0:1, :], start=True, stop=True)
                ua = gpool.tile([128, NIMG], f32, tag="ua")
                ub = gpool.tile([128, NIMG], f32, tag="ub")
                nc.vector.tensor_scalar_add(ua[:], pg[:], 256.0)
                nc.vector.tensor_scalar_add(ub[:], pg[:], 256.25)
                ui = gpool.tile([128, NIMG], i32, tag="ui")
                uf = gpool.tile([128, NIMG], f32, tag="uf")
                gm = gpool.tile([128, NIMG], f32, tag="gm")
                gmc = gpool.tile([128, NIMG], f32, tag="gmc")
                nc.vector.tensor_copy(ui[:], ua[:])
                nc.vector.tensor_copy(uf[:], ui[:])
                nc.vector.tensor_sub(gm[:], ua[:], uf[:])
                nc.vector.tensor_copy(ui[:], ub[:])
                nc.vector.tensor_copy(uf[:], ui[:])
                nc.vector.tensor_sub(gmc[:], ub[:], uf[:])
                sl = slice(t * NIMG, (t + 1) * NIMG)
                nc.scalar.activation(gsin[:, sl], gm[:], AF.Sin, scale=TWO_PI)
                nc.scalar.activation(gcos[:, sl], gmc[:], AF.Sin, scale=TWO_PI)

            o_all = opool.tile([32, NIMG * 32], f32, tag="o_all")

            # ---- main loop: groups of 4 images ----
            for g in range(NGRP):
                fi = (4 * g) // 16
                i = fi % 3
                pm1 = ps1.tile([128, 1024], f32, tag="pm1")
                scs = []
                for k in range(4):
                    jj = 4 * g + k
                    # build msuper = [m1t0|m1t1|m2t0|m2t1], each [128,128]
                    ms = mpool.tile([128, 512], fp16, tag="ms")
                    sc = scpool.tile([128, 384], fp16, tag="sc")
                    scs.append(sc)
                    for t in range(2):
                        gc = gcos[:, t * NIMG + jj: t * NIMG + jj + 1]
                        gs = gsin[:, t * NIMG + jj: t * NIMG + jj + 1]
                        xsl = slice((i * 2 + t) * 128, (i * 2 + t + 1) * 128)
                        nc.vector.tensor_scalar_mul(
                            ms[:, t * 128:(t + 1) * 128], xam[:, xsl], gc)
                        nc.vector.tensor_scalar_mul(
                            ms[:, 256 + t * 128: 384 + t * 128],
                            xbm[:, xsl], gs)
                    # adds: sc[t*192+64 : t*192+192] = [Sr|Si]_t
                    nc.gpsimd.tensor_tensor(sc[:, 64:192], ms[:, 0:128],
                                            ms[:, 256:384], op=OP.add)
                    if jj % 3 == 0:
                        nc.gpsimd.tensor_tensor(sc[:, 256:384],
                                                ms[:, 128:256],
                                                ms[:, 384:512], op=OP.add)
                    else:
                        nc.vector.tensor_tensor(sc[:, 256:384],
                                                ms[:, 128:256],
                                                ms[:, 384:512], op=OP.add)
                    # -Si blocks at cols [0:64] and [192:256]
                    sc3 = sc[:].rearrange("p (t x) -> p t x", x=192)
                    nc.vector.tensor_scalar_mul(sc3[:, :, 0:64],
                                                sc3[:, :, 128:192], -1.0)
                    # stage 1: M = Q^T S into pm1[:, k*256 + c*128 ...]
                    for c in range(2):
                        osl = slice(k * 256 + c * 128, k * 256 + c * 128 + 128)
                        for t in range(2):
                            blk = (fi * 4 + t * 2 + c) * 128
                            nc.tensor.matmul(
                                pm1[:, osl], qr[:, blk: blk + 128],
                                sc[:, t * 192 + 64: t * 192 + 192],
                                start=(t == 0), stop=False)
                            nc.tensor.matmul(
                                pm1[:, osl], qi[:, blk: blk + 128],
                                sc[:, t * 192: t * 192 + 128],
                                start=False, stop=(t == 1))
                # M psum -> sbuf fp16 (plain Act copy, 4 images at once)
                m1 = m1pool.tile([128, 1024], fp16, tag="m1")
                nc.scalar.copy(m1[:], pm1[:])
                # stage 2: field^T per image
                pm2 = ps2.tile([128, 256], f32, tag="pm2")
                for k in range(4):
                    sc = scs[k]
                    osl = slice(k * 64, k * 64 + 64)
                    for c in range(2):
                        mof = k * 256 + c * 128
                        nc.tensor.matmul(
                            pm2[:, osl], sc[:, c * 192 + 64: c * 192 + 192],
                            m1[:, mof: mof + 64],
                            start=(c == 0), stop=False)
                        nc.tensor.matmul(
                            pm2[:, osl], sc[:, c * 192: c * 192 + 128],
                            m1[:, mof + 64: mof + 128],
                            start=False, stop=(c == 1))
                # |field|^2 (Act) then r-side pair-add (Pool)
                sq = sqpool.tile([128, 256], fp16, tag="sq")
                nc.scalar.activation(sq[:], pm2[:], AF.Square)
                if g % 2 == 0:
                    a1 = a1pool.tile([128, 256], fp16, tag="a1")
                sq3 = sq[:].rearrange("p (k x) -> p k x", x=64)
                a13 = a1[:, (g % 2) * 128:(g % 2) * 128 + 128].rearrange(
                    "p (k x) -> p k x", x=32)
                nc.gpsimd.tensor_tensor(a13, sq3[:, :, 0:32],
                                        sq3[:, :, 32:64], op=OP.add)
                # sampling matmuls into pm3[:, (jj%8)*32 ...]
                if g % 2 == 0:
                    pm3 = ps3.tile([32, 256], f32, tag="pm3")
                for k in range(4):
                    jj = 4 * g + k
                    asl = slice(((g % 2) * 4 + k) * 32,
                                ((g % 2) * 4 + k) * 32 + 32)
                    nc.tensor.matmul(pm3[:, (jj % 8) * 32:(jj % 8) * 32 + 32],
                                     wser[:, i * 32:(i + 1) * 32],
                                     a1[:, asl], start=True, stop=True)
                if g % 2 == 1:
                    osl = slice((g // 2) * 256, (g // 2) * 256 + 256)
                    nc.scalar.copy(o_all[:, osl], pm3[:])

            nc.sync.dma_start(outd[:], o_all[:])
    nc.compile()
    return nc


_CACHE = {}


def _get_nc():
    if "nc" not in _CACHE:
        _CACHE["nc"] = build_nc()
    return _CACHE["nc"]


def kernel(d_obj, current_focus_dist_0, current_focus_dist_90,
           zernike_0, zernike_90, zernike_basis, aperture, wavelengths):
    from concourse.bass_utils import run_bass_kernel_spmd

    d_obj = np.asarray(d_obj, np.float32)
    zernike_0 = np.asarray(zernike_0, np.float32)
    zernike_90 = np.asarray(zernike_90, np.float32)
    basis = np.asarray(zernike_basis, np.float32)
    aperture = np.asarray(aperture, np.float32)
    lam = np.asarray(wavelengths, np.float32)
    f0 = float(current_focus_dist_0)
    f90 = float(current_focus_dist_90)

    csel, wroot = _host_consts(lam)

    # Q blocks: qr/qi [128, 24*128], blk = p6*4 + t*2 + c holds
    # Q[t*128+u, c*128+v] for (f,i) = divmod(p6, 3)
    O = np.tensordot(np.stack([zernike_0, zernike_90]),
                     basis.reshape(NZ, -1), axes=[[1], [0]])
    O = O.reshape(2, GRID, GRID).astype(np.float64)
    qr = np.empty((128, 24 * 128), np.float16)
    qi = np.empty((128, 24 * 128), np.float16)
    for f in range(2):
        for i in range(3):
            ph = 2.0 * np.pi * O[f] / float(lam[i])
            Qr = (aperture * np.cos(ph)).astype(np.float16)
            Qi = (aperture * np.sin(ph)).astype(np.float16)
            p6 = f * 3 + i
            for t in range(2):
                for c in range(2):
                    blk = (p6 * 4 + t * 2 + c) * 128
                    qr[:, blk: blk + 128] = Qr[t * 128:(t + 1) * 128,
                                               c * 128:(c + 1) * 128]
                    qi[:, blk: blk + 128] = Qi[t * 128:(t + 1) * 128,
                                               c * 128:(c + 1) * 128]

    # build coefs: xam = gc-coefs [c1|c3], xbm = gs-coefs [c2|c4]
    idx = (np.arange(GRID) + GRID // 2) % GRID
    ang = -2.0 * np.pi * np.outer(idx, idx) / GRID
    xam = np.empty((128, 6 * 128), np.float16)
    xbm = np.empty((128, 6 * 128), np.float16)
    for i in range(3):
        beta = ang[:, csel[i]]                       # [256, 64]
        cb = wroot[i] * np.cos(beta)
        sb = wroot[i] * np.sin(beta)
        a_full = np.concatenate([cb, sb], 1)          # [c1|c3] [256, 128]
        b_full = np.concatenate([-sb, cb], 1)         # [c2|c4]
        for t in range(2):
            sl = slice((i * 2 + t) * 128, (i * 2 + t + 1) * 128)
            xam[:, sl] = a_full[t * 128:(t + 1) * 128]
            xbm[:, sl] = b_full[t * 128:(t + 1) * 128]

    # sampling matrix: sums r/i halves and s-side tap pairs (0/1 entries)
    wser = np.zeros((128, 3 * 32), np.float16)
    for i in range(3):
        for q in range(32):
            for k0 in (q, 32 + q, 64 + q, 96 + q):
                wser[k0, i * 32 + q] = 1.0

    lin = np.linspace(-1.0, 1.0, GRID)
    wv = (2.0 * lin * lin - 0.5).astype(np.float32)[None, :]

    delta = np.stack([
        F_M ** 2 / (8.0 * F_NUMBER ** 2) * (1.0 / f0 - 1.0 / (d_obj + 1e-8)),
        F_M ** 2 / (8.0 * F_NUMBER ** 2) * (1.0 / f90 - 1.0 / (d_obj + 1e-8)),
    ])  # [2, 128]

    nc = _get_nc()
    in_maps = []
    for core in range(NCORES):
        erow = np.empty((1, NIMG), np.float32)
        for jj in range(NIMG):
            fi, b = jj // 16, jj % 16
            f, i = fi // 3, fi % 3
            erow[0, jj] = (delta[f, core * BPC + b] * np.sqrt(3.0)
                           / float(lam[i]))
        in_maps.append({
            "qr": qr, "qi": qi, "xam": xam, "xbm": xbm,
            "wser": wser, "erow": erow, "wv": wv,
        })
    trace = bool(_CACHE.get("trace"))
    res = run_bass_kernel_spmd(nc, in_maps, list(range(NCORES)), trace=trace)
    _CACHE["last_res"] = res
    outs = res.results
    psf0 = np.empty((BATCH, 3, FOV, FOV), np.float32)
    psf90 = np.empty((BATCH, 3, FOV, FOV), np.float32)
    eps = np.float32(1e-8 * SCALE ** 4)
    for core in range(NCORES):
        o = np.asarray(outs[core]["out"]).reshape(32, NIMG, 32)
        o = o.transpose(1, 2, 0)            # [jj, p, q]
        o = o.reshape(2, 3, BPC, FOV, FOV)  # [f, i, b, p, q]
        s = o.sum(axis=(-2, -1), keepdims=True)
        o = o / (s + eps)
        psf0[core * BPC:(core + 1) * BPC] = o[0].transpose(1, 0, 2, 3)
        psf90[core * BPC:(core + 1) * BPC] = o[1].transpose(1, 0, 2, 3)
    return psf0, psf90
